# revision 10
# baseline (speedup 1.0000x reference)
"""Causal single-head self-attention on 8 trn2 NeuronCores.

Problem: x [4, 4096, 1024] fp32, w_q/w_k/w_v [1024, 64] fp32.
  q,k,v = x @ w_{q,k,v};  y = softmax(causal(q k^T) / 8) v   -> [4, 4096, 64]

Sharding: 8 cores = 4 batches x 2 query-parity shards. Core c handles
batch b = c//2 and the query rows  h::2  (h = c%2). Interleaving the
query rows by parity makes every core's causal structure identical, so
one SPMD program serves all 8 cores; the h-dependence is folded into a
per-core causal-mask input tensor.

Per-core kernel (Bass/Tile), optimized around the ACT-exp floor:
  - x^T arrives host-pretransposed in bf16 ([E, SL]); no PE transposes.
  - kv^T = [w_k|w_v]^T x^T (bf16 matmuls, M=128); q^T column-tiled 2x
    (two 512-col blocks concurrently via tile_position=(0,0)/(0,64)).
  - own kv^T half sent in 4 chunks; pairwise AllGather; k^T duplicated
    into both SBUF partition halves (kkT) so scores can row-tile.
  - scores^T (K=64) row-tiled 2x: consecutive matmuls alternate
    tile_position (0,0)/(64,0) reading kkT/qT2 lo/hi halves. Diagonal
    tiles use trimmed N = 512-128r; the causal staircase mask is
    r-independent ([128, 512] per key-parity, multiplied post-exp).
  - exp on ACT from multi-bank PSUM groups (3/2 tiles per ACTIVATE),
    writing fp8e4 (full tiles) / bf16 (diagonal tiles).
  - AV: full tiles via fp8e4 DoubleRow (K=256: two k-tiles per matmul,
    ones-column denominator, M=65); diagonal tiles bf16 M=65.
  - finalize: PSUM->SBUF, PE transpose per 128 queries, reciprocal of
    the ones-row, scale, one batched DMA out per q-block.
"""
import sys

sys.path.insert(0, "/opt/trn_rl_repo")

import numpy as np

import concourse.bass as bass
import concourse.mybir as mybir
from concourse import bacc
from concourse.tile import TileContext
from concourse.masks import make_identity

F32 = mybir.dt.float32
BF16 = mybir.dt.bfloat16
FP8 = mybir.dt.float8e4

B, S, E, D = 4, 4096, 1024, 64
NCORES = 8
SL = S // 2          # local q rows per core (parity shard)
NE = E // 128        # 8 E-chunks
NKT = S // 128       # 32 gathered k-tiles (0:16 even parity, 16:32 odd)
QB = 512             # local q-block size (spans 1024 global rows)
NQB = SL // QB       # 4 local q-blocks
USE_DR = True        # fp8e4 DoubleRow for full-tile AV matmuls


def build_nc(iters=1):
    from contextlib import ExitStack

    nc = bacc.Bacc(trn_type="TRN2", num_devices=NCORES)
    xt = nc.declare_dram_parameter("xt", [E, SL], BF16, isOutput=False)
    wkv = nc.declare_dram_parameter("wkv", [E, 128], BF16, isOutput=False)
    wq = nc.declare_dram_parameter("wq", [E, D], BF16, isOutput=False)
    masks = nc.declare_dram_parameter("masks", [2, 128, QB], BF16, isOutput=False)
    y_out = nc.declare_dram_parameter("y", [SL, D], F32, isOutput=True)
    kv_snd = nc.dram_tensor("kv_snd", [4, 128, QB], BF16)
    kv_gat = nc.dram_tensor("kv_gat", [4, 2, 128, QB], BF16)
    pair_groups = [[2 * p, 2 * p + 1] for p in range(NCORES // 2)]

    ET8 = FP8 if USE_DR else BF16

    with TileContext(nc) as tc:
        with tc.tile_pool(name="singles", bufs=1) as singles, \
             tc.tile_pool(name="big", bufs=1) as big:
            ident = singles.tile([128, 128], F32)
            make_identity(nc, ident)
            ident_bf = singles.tile([128, 128], BF16)
            nc.vector.tensor_copy(out=ident_bf, in_=ident)
            # uniform exp shift (softmax-invariant) keeping fp8e4 in range
            exp_bias = singles.tile([128, 1], F32)
            nc.vector.memset(exp_bias, -2.5)

            wkv_sb = singles.tile([128, NE, 128], BF16)
            nc.sync.dma_start(
                out=wkv_sb, in_=wkv.rearrange("(c p) m -> p c m", p=128)
            )
            wq_sb = singles.tile([128, NE, D], BF16)
            nc.sync.dma_start(
                out=wq_sb, in_=wq.rearrange("(c p) m -> p c m", p=128)
            )
            masks_sb = singles.tile([128, 2, QB], BF16)
            nc.sync.dma_start(out=masks_sb, in_=masks.rearrange("k p c -> p k c"))

            # persistent SBUF working set
            kkT = big.tile([128, S], BF16)       # k^T duplicated in both halves
            qT2 = big.tile([128, SL], BF16)      # q^T duplicated in both halves
            vT = big.tile([64, S], BF16)         # v^T gathered
            v_bf = big.tile([128, NKT // 2, 2, 80], BF16)   # v rows + ones col
            v_f8 = big.tile([128, NKT // 2, 2, 80], FP8)
            eT8 = big.tile([128, NKT, QB], ET8)  # full-tile exp'd scores
            kvT_own = big.tile([128, SL], BF16)

            hint = (
                mybir.EngineType.PE,
                mybir.EngineType.DVE,
                mybir.EngineType.Activation,
                mybir.EngineType.SP,
            )
            loop_ctx = ExitStack()
            if iters > 1:
                loop_ctx.enter_context(tc.For_i(0, iters, 1, hint_engines=hint))

            # ---------------- phase 1: projections ----------------
            with tc.tile_pool(name="ph1ps", bufs=1, space="PSUM") as ps1, \
                 tc.tile_pool(name="ph1sb", bufs=1) as sb1:
                xts = [
                    sb1.tile([128, NE, QB], BF16, tag=f"xt{sb}", name=f"xt{sb}")
                    for sb in range(4)
                ]
                xt_r = xt.rearrange("(c p) s -> p c s", p=128)
                for sb in range(4):
                    sl = slice(sb * QB, (sb + 1) * QB)
                    nc.sync.dma_start(out=xts[sb], in_=xt_r[:, :, sl])
                    pkv = ps1.tile([128, QB], F32, tag="pkv", bufs=2, name="pkv")
                    for c in range(NE):
                        nc.tensor.matmul(
                            pkv, wkv_sb[:, c, :], xts[sb][:, c, :],
                            start=(c == 0), stop=(c == NE - 1),
                        )
                    nc.vector.tensor_copy(out=kvT_own[:, sl], in_=pkv)
                    nc.sync.dma_start(out=kv_snd[sb], in_=kvT_own[:, sl])
                # q, column-tiled 2x over block pairs
                for pr in range(2):
                    slA = slice((2 * pr) * QB, (2 * pr + 1) * QB)
                    slB = slice((2 * pr + 1) * QB, (2 * pr + 2) * QB)
                    pq = ps1.tile([128, QB], F32, tag="pq", bufs=2, name="pq")
                    for c in range(NE):
                        nc.tensor.matmul(
                            pq[0:64, :], wq_sb[:, c, :], xts[2 * pr][:, c, :],
                            start=(c == 0), stop=(c == NE - 1),
                            tile_position=(0, 0),
                        )
                        nc.tensor.matmul(
                            pq[64:128, :], wq_sb[:, c, :],
                            xts[2 * pr + 1][:, c, :],
                            start=(c == 0), stop=(c == NE - 1),
                            tile_position=(0, 64),
                        )
                    nc.vector.tensor_copy(out=qT2[0:64, slA], in_=pq[0:64, :])
                    nc.vector.tensor_copy(out=qT2[64:128, slA], in_=pq[0:64, :])
                    nc.vector.tensor_copy(out=qT2[0:64, slB], in_=pq[64:128, :])
                    nc.vector.tensor_copy(out=qT2[64:128, slB], in_=pq[64:128, :])

            if iters > 1:
                loop_ctx.close()
            for c in range(4):
                nc.gpsimd.collective_compute(
                    "AllGather", mybir.AluOpType.bypass,
                    replica_groups=pair_groups,
                    ins=[kv_snd[c]], outs=[kv_gat[c]],
                )
            if iters > 1:
                loop_ctx.enter_context(tc.For_i(0, iters, 1, hint_engines=hint))

            # gathered loads: k^T duplicated into both halves; v^T
            for g in range(2):
                gsl = slice(g * SL, (g + 1) * SL)
                kk_src = kv_gat[:, g, 0:64, :].rearrange("c p s -> p c s")
                kk_dst = kkT[:, gsl].rearrange("p (c s) -> p c s", c=4)
                nc.sync.dma_start(out=kk_dst[0:64], in_=kk_src)
                nc.sync.dma_start(out=kk_dst[64:128], in_=kk_src)
                nc.sync.dma_start(
                    out=vT[:, gsl].rearrange("p (c s) -> p c s", c=4),
                    in_=kv_gat[:, g, 64:128, :].rearrange("c p s -> p c s"),
                )

            # ---------------- phase 2: v tiles + attention ----------------
            with tc.tile_pool(name="ph2vps", bufs=1, space="PSUM") as psv:
                nc.vector.memset(v_bf, 1.0)
                for kt in range(NKT):
                    pvt = psv.tile([128, 64], BF16, tag="pvt", bufs=2,
                                   name="pvt")
                    nc.tensor.transpose(
                        pvt, vT[:, kt * 128:(kt + 1) * 128],
                        ident_bf[0:64, 0:64],
                    )
                    nc.vector.tensor_copy(
                        out=v_bf[:, kt // 2, kt % 2, 0:64], in_=pvt
                    )
                if USE_DR:
                    nc.vector.tensor_copy(out=v_f8, in_=v_bf)

            with tc.tile_pool(name="ph2ps", bufs=1, space="PSUM") as ps2, \
                 tc.tile_pool(name="ph2sb", bufs=1) as sb2:
                rt = 0  # row-tile position alternator for scores

                def score_mm(ps_out, t, csl):
                    nonlocal rt
                    lo = (rt % 2 == 0)
                    rt += 1
                    h0, h1 = (0, 64) if lo else (64, 128)
                    nc.tensor.matmul(
                        ps_out,
                        kkT[h0:h1, t * 128:(t + 1) * 128],
                        qT2[h0:h1, csl],
                        start=True, stop=True,
                        tile_position=(h0, 0),
                    )

                for j in range(NQB):
                    qsl = slice(j * QB, (j + 1) * QB)
                    y_ps = ps2.tile([65, QB], F32, tag=f"y{j % 2}", bufs=1,
                                    name="y_ps")
                    av = []  # deferred AV matmuls (lhsT, rhs, out_slice)

                    # --- scores + exp: full tiles, grouped 3/2 ---
                    runs = [list(range(4 * j)),
                            [16 + t for t in range(4 * j)]]
                    sgi = 0
                    for run in runs:
                        while run:
                            gsz = 3 if sgi % 2 == 0 else 2
                            gsz = min(gsz, len(run))
                            grp = [run.pop(0) for _ in range(gsz)]
                            sg = ps2.tile(
                                [128, 3 if sgi % 2 == 0 else 2, QB], F32,
                                tag=f"sg{sgi % 2}", bufs=1, name="sg",
                            )
                            sgi += 1
                            for i, t in enumerate(grp):
                                score_mm(sg[:, i, :], t, qsl)
                            t0 = grp[0]
                            nc.scalar.activation(
                                out=eT8[:, t0:t0 + gsz, :],
                                in_=sg[:, 0:gsz, :],
                                func=mybir.ActivationFunctionType.Exp,
                                scale=0.125, bias=exp_bias,
                            )
                    if USE_DR:
                        for p in [t // 2 for t in (list(range(0, 4 * j, 2))
                                                   + list(range(16, 16 + 4 * j, 2)))]:
                            av.append((
                                v_f8[:, p, :, 0:65],
                                eT8[:, 2 * p:2 * p + 2, :],
                                slice(0, QB), "dr",
                            ))
                    else:
                        for t in (list(range(4 * j))
                                  + [16 + t for t in range(4 * j)]):
                            av.append((
                                v_bf[:, t // 2, t % 2, 0:65],
                                eT8[:, t, :],
                                slice(0, QB), "bf",
                            ))

                    # --- scores + exp: diagonal tiles, trimmed ---
                    for r in range(4):
                        nr = QB - 128 * r
                        csl = slice(j * QB + 128 * r, (j + 1) * QB)
                        sg = ps2.tile(
                            [128, 3 if sgi % 2 == 0 else 2, QB], F32,
                            tag=f"sg{sgi % 2}", bufs=1, name="sgd",
                        )
                        sgi += 1
                        score_mm(sg[:, 0, 0:nr], 4 * j + r, csl)
                        score_mm(sg[:, 1, 0:nr], 16 + 4 * j + r, csl)
                        ed = sb2.tile([128, 2, QB], BF16, tag="ed", bufs=2,
                                      name="ed")
                        nc.scalar.activation(
                            out=ed[:, :, 0:nr], in_=sg[:, 0:2, 0:nr],
                            func=mybir.ActivationFunctionType.Exp,
                            scale=0.125, bias=exp_bias,
                        )
                        etd = sb2.tile([128, 2, QB], BF16, tag=f"etd{r}",
                                       bufs=2, name="etd")
                        nc.vector.tensor_mul(
                            etd[:, :, 0:nr], ed[:, :, 0:nr],
                            masks_sb[:, :, 0:nr],
                        )
                        for kp in range(2):
                            av.append((
                                v_bf[:, (4 * j + r) // 2 + 8 * kp,
                                     (4 * j + r) % 2, 0:65],
                                etd[:, kp, 0:nr],
                                slice(128 * r, QB), "bf",
                            ))

                    # --- AV accumulation into y_ps ---
                    for i, (lhsT, rhs, osl, kind) in enumerate(av):
                        nc.tensor.matmul(
                            y_ps[:, osl], lhsT, rhs,
                            start=(i == 0), stop=(i == len(av) - 1),
                            perf_mode=(mybir.MatmulPerfMode.DoubleRow
                                       if kind == "dr" else None),
                        )

                    # --- finalize q-block ---
                    ysb = sb2.tile([65, QB], F32, tag="ysb", bufs=2,
                                   name="ysb")
                    nc.vector.tensor_copy(out=ysb, in_=y_ps)
                    stg = sb2.tile([128, 4, 64], F32, tag="stg", bufs=2,
                                   name="stg")
                    for qq in range(QB // 128):
                        pyt = ps2.tile([128, 65], F32, tag="pyt", bufs=1,
                                       name="pyt")
                        nc.tensor.transpose(
                            pyt, ysb[:, qq * 128:(qq + 1) * 128],
                            ident[0:65, 0:65],
                        )
                        rec = sb2.tile([128, 1], F32, tag="rec", bufs=2,
                                       name="rec")
                        nc.vector.reciprocal(rec, pyt[:, 64:65])
                        nc.vector.tensor_scalar_mul(
                            stg[:, qq, :], pyt[:, 0:64], rec
                        )
                    nc.sync.dma_start(
                        out=y_out.rearrange("(j q p) d -> p j q d", j=NQB,
                                            q=QB // 128)[:, j],
                        in_=stg,
                    )
            loop_ctx.close()
    nc.finalize()
    return nc


class _Runner:
    """Compile once; re-execute the sharded program with cached jit."""

    def __init__(self, nc):
        import jax
        from jax.sharding import Mesh, PartitionSpec
        from jax.experimental.shard_map import shard_map
        from concourse import bass2jax, mybir as _mb

        bass2jax.install_neuronx_cc_hook()
        self.nc = nc
        self._jax = jax
        self._bass2jax = bass2jax

        partition_name = (
            nc.partition_id_tensor.name if nc.partition_id_tensor else None
        )
        in_names, out_names, out_avals, zero_shapes = [], [], [], []
        for alloc in nc.m.functions[0].allocations:
            if not isinstance(alloc, _mb.MemoryLocationSet):
                continue
            name = alloc.memorylocations[0].name
            if alloc.kind == "ExternalInput":
                if name != partition_name:
                    in_names.append(name)
            elif alloc.kind == "ExternalOutput":
                shape = tuple(alloc.tensor_shape)
                dtype = _mb.dt.np(alloc.dtype)
                out_names.append(name)
                out_avals.append(jax.core.ShapedArray(shape, dtype))
                zero_shapes.append((shape, dtype))
        self.in_names = list(in_names)
        self.out_names = out_names
        self.zero_shapes = zero_shapes
        n_params = len(in_names)
        n_outs = len(out_avals)
        all_in_names = list(in_names) + list(out_names)
        if partition_name is not None:
            all_in_names.append(partition_name)
        donate = tuple(range(n_params, n_params + n_outs))

        def _body(*args):
            operands = list(args)
            if partition_name is not None:
                operands.append(bass2jax.partition_id_tensor())
            outs = bass2jax._bass_exec_p.bind(
                *operands,
                out_avals=tuple(out_avals),
                in_names=tuple(all_in_names),
                out_names=tuple(out_names),
                lowering_input_output_aliases=(),
                sim_require_finite=True,
                sim_require_nnan=True,
                nc=nc,
            )
            return tuple(outs)

        devices = jax.devices()[:NCORES]
        mesh = Mesh(np.asarray(devices), ("core",))
        in_specs = (PartitionSpec("core"),) * (n_params + n_outs)
        out_specs = (PartitionSpec("core"),) * n_outs
        self.sharded = jax.jit(
            shard_map(_body, mesh=mesh, in_specs=in_specs, out_specs=out_specs,
                      check_rep=False),
            donate_argnums=donate, keep_unused=True,
        )
        self.mesh = mesh
        self.pspec = PartitionSpec("core")

    def put_inputs(self, in_maps):
        import jax
        from jax.sharding import NamedSharding
        sh = NamedSharding(self.mesh, self.pspec)
        arrs = []
        for name in self.in_names:
            cat = np.concatenate([np.asarray(m[name]) for m in in_maps], axis=0)
            arrs.append(jax.device_put(cat, sh))
        return arrs

    def zeros(self):
        import jax
        from jax.sharding import NamedSharding
        sh = NamedSharding(self.mesh, self.pspec)
        return [
            jax.device_put(np.zeros((NCORES * s[0], *s[1:]), d), sh)
            for (s, d) in self.zero_shapes
        ]

    def run(self, dev_inputs):
        outs = self.sharded(*dev_inputs, *self.zeros())
        return outs

    def results(self, outs):
        out = {}
        for i, name in enumerate(self.out_names):
            a = np.asarray(outs[i])
            out[name] = a.reshape(NCORES, a.shape[0] // NCORES, *a.shape[1:])
        return out


_RUNNER = None


def _get_runner():
    global _RUNNER
    if _RUNNER is None:
        _RUNNER = _Runner(build_nc())
    return _RUNNER


def _make_masks(h: int) -> np.ndarray:
    # Diagonal-tile staircase mask in tile-local coordinates: tile col 0
    # is the first query column the tile can attend from, so the mask is
    # independent of the sub-tile index r. Cols 128: are fully valid.
    # key-in-band: 2p + kp; query: 2c + h (c tile-local col, p partition).
    from ml_dtypes import bfloat16
    out = np.ones((2, 128, QB), dtype=np.float32)
    p = np.arange(128)[:, None]
    c = np.arange(128)[None, :]
    for kp in range(2):
        out[kp, :, 0:128] = (2 * p + kp <= 2 * c + h)
    return out.astype(bfloat16)


def make_in_maps(x, w_q, w_k, w_v):
    from ml_dtypes import bfloat16
    x = np.asarray(x, dtype=np.float32)
    w_q = np.asarray(w_q, dtype=np.float32).astype(bfloat16)
    w_k = np.asarray(w_k, dtype=np.float32)
    w_v = np.asarray(w_v, dtype=np.float32)
    wkv = np.ascontiguousarray(
        np.concatenate([w_k, w_v], axis=1)).astype(bfloat16)
    masks = [_make_masks(0), _make_masks(1)]

    in_maps = []
    for c in range(NCORES):
        b, h = c // 2, c % 2
        in_maps.append({
            "xt": np.ascontiguousarray(x[b, h::2].T).astype(bfloat16),
            "wkv": wkv,
            "wq": w_q,
            "masks": masks[h],
        })
    return in_maps


def kernel(x, w_q, w_k, w_v):
    runner = _get_runner()
    in_maps = make_in_maps(x, w_q, w_k, w_v)
    dev_inputs = runner.put_inputs(in_maps)
    outs = runner.results(runner.run(dev_inputs))

    y = np.empty((B, S, D), dtype=np.float32)
    for c in range(NCORES):
        b, h = c // 2, c % 2
        y[b, h::2, :] = outs["y"][c]
    return y
